# revision 1
# baseline (speedup 1.0000x reference)
"""Distributed Trainium2 kernel for GQA attention (nn_Attention_76845554860188).

B=1, S=2048, D=1024, NH=16, NKV=4, HD=64, causal, RoPE, 8 NeuronCores.

Sharding: tensor-parallel over heads. Core c owns q-heads {2c, 2c+1} and their
(shared, GQA) kv-head c//2. Each core projects Q/K/V for all 2048 positions and
runs causal attention for its 2 heads, flash-style: per 512-wide q-chunk, the
PV matmul accumulates in PSUM block-by-block as scores come out of exp, so the
probability tiles stay small and the final chunk's tail is one PV matmul.

Scores use a full-array (128-deep) contraction that folds the q-side RoPE in:
  score = krot.q_rot = [krot; M^T krot] . [q*cos; q*sin]
so q is never explicitly rotated (no q-rope matmuls, no per-head copies) and
the PE array runs with all 128 rows active. The K side builds
kst = [krot; M^T krot] with two small 64-contraction matmuls per seq window
using host-prepared [I | M] / [M^T | I] operators.

Output redistribution uses two AllToAlls on a strided q-block assignment: core
c owns q-128-blocks {c, 8+c}. A2A#1 (blocks 0-7) fires after chunk 1 and
overlaps attention of chunks 2-3 together with the first half of the output
projection; only A2A#2 + the second half-projection sit on the tail.

The softmax denominator comes free as a ones column appended to V in the PV
matmul. exp() runs once per k-block over both heads ([128, 2, w]) on ScalarE
with the 1/sqrt(64) scale folded in; no max-subtraction is needed (logits are
O(5) for unit-scale inputs, far from bf16 overflow).
"""

import sys

sys.path.insert(0, "/opt/trn_rl_repo")

import numpy as np
import ml_dtypes

import concourse.bass as bass
import concourse.mybir as mybir
import concourse.tile as tile
from concourse import bacc
from concourse.bass_utils import run_bass_kernel_spmd

BF16 = mybir.dt.bfloat16
F32 = mybir.dt.float32

B, S, D = 1, 2048, 1024
NH, NKV, HD = 16, 4, 64
NC_CORES = 8
HPC = NH // NC_CORES  # q heads per core = 2
NDC = D // 128  # d chunks = 8
NSB = S // 128  # 128-wide seq blocks = 16
NCH = S // 512  # 512-wide seq chunks = 4
HALF = HD // 2  # 32

np_bf16 = ml_dtypes.bfloat16


def build_graph():
    nc = bacc.Bacc(
        "TRN2", target_bir_lowering=False, debug=False, num_devices=NC_CORES
    )

    # ---- DRAM parameters (per-core shards supplied by host) ----
    xT_e = nc.dram_tensor("xT", [D, S], BF16, kind="ExternalInput")
    wq_e = nc.dram_tensor("wq", [128, NDC, HPC * HD], BF16, kind="ExternalInput")
    wkv_e = nc.dram_tensor("wkv", [128, NDC, 2 * HD], BF16, kind="ExternalInput")
    wo_e = nc.dram_tensor("wo", [128, NDC, D], BF16, kind="ExternalInput")
    c2_e = nc.dram_tensor("c2", [128, S], BF16, kind="ExternalInput")
    s2_e = nc.dram_tensor("s2", [128, S], BF16, kind="ExternalInput")
    ka_e = nc.dram_tensor("ka", [64, 128], BF16, kind="ExternalInput")
    kb_e = nc.dram_tensor("kb", [64, 128], BF16, kind="ExternalInput")
    idm_e = nc.dram_tensor("idm", [128, 128], BF16, kind="ExternalInput")
    tri2_e = nc.dram_tensor("tri2", [128, 2 * 128], BF16, kind="ExternalInput")
    # rows [0:128] = q-block c, rows [128:256] = q-block 8+c
    out_e = nc.dram_tensor("out", [2 * 128, D], BF16, kind="ExternalOutput")

    # A2A bounce buffers: slot j = the 128-wide q-block destined for core j
    send_d = [nc.dram_tensor(f"a2a_send{i}", [NC_CORES, 128, 128], BF16)
              for i in range(2)]
    recv_d = [nc.dram_tensor(f"a2a_recv{i}", [NC_CORES, 128, 128], BF16)
              for i in range(2)]
    # tiny warmup collective: absorbs the entry barrier + collective-stream
    # setup during the preamble so the real A2As run at steady-state cost
    wup_s = nc.dram_tensor("wup_s", [1, 64], BF16)
    wup_r = nc.dram_tensor("wup_r", [NC_CORES, 1, 64], BF16, addr_space="Shared")

    with tile.TileContext(nc) as tc:
        _body(nc, tc, xT_e, wq_e, wkv_e, wo_e, c2_e, s2_e, ka_e, kb_e, idm_e,
              tri2_e, out_e, send_d, recv_d, wup_s, wup_r)

    nc.compile()
    return nc


def _body(nc, tc, xT_e, wq_e, wkv_e, wo_e, c2_e, s2_e, ka_e, kb_e, idm_e,
          tri2_e, out_e, send_d, recv_d, wup_s, wup_r):
    from contextlib import ExitStack

    ctx = ExitStack()
    with ctx:
        consts = ctx.enter_context(tc.tile_pool(name="consts", bufs=1))
        work = ctx.enter_context(tc.tile_pool(name="work", bufs=1))
        rope_cm = tc.tile_pool(name="rope", bufs=1)
        rope = rope_cm.__enter__()
        psum_cm = tc.tile_pool(name="psum", bufs=2, space="PSUM")
        psum = psum_cm.__enter__()

        # warmup collective, first in program order
        wup_sb = consts.tile([1, 64], BF16, tag="wup")
        nc.vector.memset(wup_sb[:], 0.0)
        nc.sync.dma_start(out=wup_s.ap(), in_=wup_sb[:])
        nc.gpsimd.collective_compute(
            "AllGather",
            mybir.AluOpType.bypass,
            replica_groups=[list(range(NC_CORES))],
            ins=[wup_s.ap().opt()],
            outs=[wup_r.ap().opt()],
        )

        # ---- load inputs needed by the preamble (Wo is loaded later) ----
        wq_sb = consts.tile([128, NDC, HPC * HD], BF16, tag="wq")
        nc.scalar.dma_start(out=wq_sb[:], in_=wq_e.ap())
        wkv_sb = consts.tile([128, NDC, 2 * HD], BF16, tag="wkv")
        nc.scalar.dma_start(out=wkv_sb[:], in_=wkv_e.ap())
        xT_sb = consts.tile([128, NDC, S], BF16, tag="xT")
        for i in range(NDC):
            eng = nc.sync if i % 2 == 0 else nc.scalar
            eng.dma_start(out=xT_sb[:, i, :], in_=xT_e[128 * i : 128 * (i + 1), :])
        c2_sb = rope.tile([128, S], BF16, tag="c2")
        nc.sync.dma_start(out=c2_sb[:], in_=c2_e[:, :])
        s2_sb = rope.tile([128, S], BF16, tag="s2")
        nc.scalar.dma_start(out=s2_sb[:], in_=s2_e[:, :])
        ka_sb = rope.tile([64, 128], BF16, tag="ka")
        nc.sync.dma_start(out=ka_sb[:], in_=ka_e[:, :])
        kb_sb = rope.tile([64, 128], BF16, tag="kb")
        nc.sync.dma_start(out=kb_sb[:], in_=kb_e[:, :])
        idm_sb = consts.tile([128, 128], BF16, tag="idm")
        nc.sync.dma_start(out=idm_sb[:], in_=idm_e[:, :])
        tri2_sb = consts.tile([128, 2, 128], BF16, tag="tri2")
        nc.sync.dma_start(
            out=tri2_sb[:], in_=tri2_e.ap().rearrange("p (h n) -> p h n", h=2)
        )

        # ---- Q/KV projections -> PSUM f32 [128, 2048] ----
        # interleaved per d-chunk so both finish right after the last xT DMA
        q_ps = psum.tile([128, S], F32, tag="big")
        kv_ps = psum.tile([128, S], F32, tag="big")
        for i in range(NDC):
            for n in range(NCH):
                nc.tensor.matmul(
                    q_ps[:, 512 * n : 512 * (n + 1)],
                    lhsT=wq_sb[:, i, :],
                    rhs=xT_sb[:, i, 512 * n : 512 * (n + 1)],
                    start=(i == 0),
                    stop=(i == NDC - 1),
                )
            for n in range(NCH):
                nc.tensor.matmul(
                    kv_ps[:, 512 * n : 512 * (n + 1)],
                    lhsT=wkv_sb[:, i, :],
                    rhs=xT_sb[:, i, 512 * n : 512 * (n + 1)],
                    start=(i == 0),
                    stop=(i == NDC - 1),
                )

        # copy projections to SBUF bf16: frees the proj PSUM for kst and the
        # attention pools, and lets the q-side multiplies run in 16-bit DVE
        # mode. qsb (ScalarE) goes first: it releases q_ps, whose PSUM slot
        # the kst accumulator reuses.
        qsb = rope.tile([128, S], BF16, tag="qsb")
        nc.scalar.copy(out=qsb[:], in_=q_ps[:])
        kvsb = rope.tile([128, S], BF16, tag="kvsb")
        nc.scalar.copy(out=kvsb[:], in_=kv_ps[:])

        # ---- K: kst = [krot; M^T krot] via kA = [I | M], kB = [M^T | I] ----
        # kc/ks read kv_ps straight from PSUM so they don't wait on kvsb
        kc_sb = rope.tile([64, S], BF16, tag="kc")
        nc.vector.tensor_tensor(
            out=kc_sb[:], in0=kv_ps[0:64, :], in1=c2_sb[0:64, :],
            op=mybir.AluOpType.mult,
        )
        ks_sb = rope.tile([64, S], BF16, tag="ks")
        nc.vector.tensor_tensor(
            out=ks_sb[:], in0=kv_ps[0:64, :], in1=s2_sb[0:64, :],
            op=mybir.AluOpType.mult,
        )
        kst_ps = psum.tile([128, S], F32, tag="big")
        for n in range(NCH):
            sl = slice(512 * n, 512 * (n + 1))
            nc.tensor.matmul(
                kst_ps[:, sl], lhsT=ka_sb[:], rhs=kc_sb[:, sl],
                start=True, stop=False,
            )
            nc.tensor.matmul(
                kst_ps[:, sl], lhsT=kb_sb[:], rhs=ks_sb[:, sl],
                start=False, stop=True,
            )
        kst_sb = work.tile([128, S], BF16, tag="kst")
        for n in range(NCH):
            sl = slice(512 * n, 512 * (n + 1))
            nc.scalar.copy(out=kst_sb[:, sl], in_=kst_ps[:, sl])

        # ---- q-side RoPE halves: qcs[h] = [q_h * cos; q_h * sin] ----
        # column-halved so chunk-0 scores only wait for the first half
        qcs = work.tile([128, HPC, S], BF16, tag="qcs")

        def qcs_half(cw):
            sl = slice(1024 * cw, 1024 * (cw + 1))
            for h in range(HPC):
                nc.vector.tensor_tensor(
                    out=qcs[0:64, h, sl], in0=qsb[64 * h : 64 * (h + 1), sl],
                    in1=c2_sb[64 * h : 64 * (h + 1), sl],
                    op=mybir.AluOpType.mult,
                )
                nc.vector.tensor_tensor(
                    out=qcs[64:128, h, sl], in0=qsb[64 * h : 64 * (h + 1), sl],
                    in1=s2_sb[64 * h : 64 * (h + 1), sl],
                    op=mybir.AluOpType.mult,
                )

        qcs_half(0)

        # ---- V transpose: kvsb rows 64:128 -> V blocks [128, 64] + ones ----
        vext_sb = work.tile([128, NSB, HD + 1], BF16, tag="vext")
        nc.vector.memset(vext_sb[:, :, HD : HD + 1], 1.0)
        vt_ps = psum.tile([128, NSB, HD], BF16, tag="big")
        for b in range(NSB):
            nc.tensor.transpose(
                vt_ps[:, b, :], kvsb[64:128, 128 * b : 128 * (b + 1)],
                idm_sb[64:128, 64:128],
            )
        nc.vector.tensor_copy(out=vext_sb[:, :, 0:HD], in_=vt_ps[:])
        qcs_half(1)

        # release RoPE temporaries and the projection-phase PSUM pool; the
        # attention phase needs st(4) + ot(2) + op(2) = 8 PSUM banks
        rope_cm.__exit__(None, None, None)
        psum_cm.__exit__(None, None, None)
        ptp = ctx.enter_context(tc.tile_pool(name="pt", bufs=2, space="SBUF"))
        psa_cm = tc.tile_pool(name="psa", bufs=2, space="PSUM")
        psa = psa_cm.__enter__()

        # Wo load: deferred past the preamble so it doesn't compete with xT
        wo_sb = consts.tile([128, NDC, D], BF16, tag="wo")
        for i in range(NDC):
            eng = nc.sync if i % 2 == 0 else nc.scalar
            eng.dma_start(out=wo_sb[:, i, :], in_=wo_e[:, i, :])

        scale = 1.0 / np.sqrt(HD)

        def oproj(half):
            """Output projection for this core's q-block {c + 8*half}."""
            at_sb = work.tile([128, NC_CORES, 128], BF16, tag="at", bufs=2,
                              name=f"at{half}")
            nc.sync.dma_start(
                out=at_sb[:],
                in_=recv_d[half].ap().rearrange("s p n -> p s n"),
            )
            ou_sb = work.tile([128, D], BF16, tag="ou", bufs=2, name=f"ou{half}")
            for dn in range(2):
                op_ps = psa.tile([128, 512], F32, tag="op", bufs=1,
                                 name=f"op{half}_{dn}")
                for j in range(NC_CORES):
                    nc.tensor.matmul(
                        op_ps[:, :],
                        lhsT=at_sb[:, j, :],
                        rhs=wo_sb[:, j, 512 * dn : 512 * (dn + 1)],
                        start=(j == 0),
                        stop=(j == NC_CORES - 1),
                    )
                nc.vector.tensor_copy(
                    out=ou_sb[:, 512 * dn : 512 * (dn + 1)], in_=op_ps[:]
                )
            nc.scalar.dma_start(
                out=out_e.ap()[128 * half : 128 * (half + 1), :], in_=ou_sb[:]
            )

        # ---- attention: flash-style per 512-wide q-chunk. PV trails the
        # scores by one k-block so the Tensor stream never head-of-line
        # blocks on the previous chunk's ot release or the current exp.
        def pv(kc, b, nb, pt):
            cbase = 512 * kc
            q0 = max(cbase, 128 * b)
            w = cbase + 512 - q0
            for h in range(HPC):
                nc.tensor.matmul(
                    ot_h[h][:, q0 - cbase : 512],
                    lhsT=vext_sb[:, b, :],
                    rhs=pt[:, h, 0:w],
                    start=(b == 0),
                    stop=(b == nb - 1),
                )

        for kc in range(NCH):
            cbase = 512 * kc
            nb = 4 * kc + 4  # k-blocks participating in this chunk
            ot_h = [
                psa.tile([HD + 1, 512], F32, tag="ot", bufs=3, name=f"ot{kc}_{h}")
                for h in range(HPC)
            ]
            prev_pt = None
            for b in range(nb):
                q0 = max(cbase, 128 * b)
                w = cbase + 512 - q0
                kb = kst_sb[:, 128 * b : 128 * (b + 1)]
                st_ps = psa.tile([128, 2, 512], F32, tag="st", bufs=2)
                for h in range(HPC):
                    nc.tensor.matmul(
                        st_ps[:, h, 0:w],
                        lhsT=kb,
                        rhs=qcs[:, h, q0 : q0 + w],
                        start=True,
                        stop=True,
                    )
                pt = ptp.tile([128, 2, 512], BF16, tag="pt", bufs=3)
                nc.scalar.activation(
                    out=pt[:, :, 0:w],
                    in_=st_ps[:, :, 0:w],
                    func=mybir.ActivationFunctionType.Exp,
                    scale=scale,
                )
                if 128 * b >= cbase:
                    # diagonal block: mask the leading 128x128 (sq < sk -> 0)
                    nc.vector.tensor_tensor(
                        out=pt[:, :, 0:128],
                        in0=pt[:, :, 0:128],
                        in1=tri2_sb[:],
                        op=mybir.AluOpType.mult,
                    )
                if prev_pt is not None:
                    pv(kc, b - 1, nb, prev_pt)
                prev_pt = pt
            pv(kc, nb - 1, nb, prev_pt)

            # normalize: row HD of ot is the softmax denominator. Per-head
            # chains so h0 starts as soon as its PV accumulation stops.
            stg = work.tile([128, 512], BF16, tag="stg", bufs=2, name=f"stg{kc}")
            for h in range(HPC):
                den_sb = work.tile([1, 512], F32, tag="den", bufs=4)
                nc.vector.tensor_copy(out=den_sb[:], in_=ot_h[h][HD : HD + 1, :])
                rec_sb = work.tile([1, 512], F32, tag="rec", bufs=4)
                nc.vector.reciprocal_approx_fast(out=rec_sb[:], in_=den_sb[:])
                bcr_sb = work.tile([HD, 512], F32, tag="bcr", bufs=4)
                nc.gpsimd.partition_broadcast(bcr_sb[:], rec_sb[:])
                nc.vector.tensor_tensor(
                    out=stg[64 * h : 64 * (h + 1), :],
                    in0=ot_h[h][0:HD, :],
                    in1=bcr_sb[:],
                    op=mybir.AluOpType.mult,
                )
            # stage chunk kc = q-blocks {4kc..4kc+3}: slots of A2A kc//2.
            # A2A#2's sends go on the scalar queue: Tile orders them after
            # A2A#1 completes, and on the sync queue that wait would
            # head-of-line block the at0/at1 receive DMAs behind it.
            half = kc // 2
            eng = nc.sync if half == 0 else nc.scalar
            eng.dma_start(
                out=send_d[half].ap()[4 * (kc % 2) : 4 * (kc % 2) + 4]
                .rearrange("j p n -> p j n"),
                in_=stg[:],
            )
            if kc % 2 == 1:
                nc.gpsimd.collective_compute(
                    "AllToAll",
                    mybir.AluOpType.bypass,
                    replica_groups=[list(range(NC_CORES))],
                    ins=[send_d[half].ap().opt()],
                    outs=[recv_d[half].ap().opt()],
                )
        # both output projections after every collective is triggered, so the
        # chunk-3 normalize -> A2A#2 chain never serializes behind them
        oproj(0)
        oproj(1)

        psa_cm.__exit__(None, None, None)


# ---------------- host side ----------------

_CACHE = {}


def _prep_consts():
    # M: signed half-swap for one 64-wide head (rotate-half convention)
    M = np.zeros((64, 64), np.float32)
    for j in range(HALF):
        M[j, HALF + j] = -1.0
        M[HALF + j, j] = 1.0
    I = np.eye(64, dtype=np.float32)
    # kst = [[I],[M^T]] @ kc + [[M],[I]] @ ks -> lhsT operators [64, 128]
    ka = np.concatenate([I, M], axis=1).astype(np_bf16)
    kb = np.concatenate([M.T, I], axis=1).astype(np_bf16)
    idm = np.eye(128, dtype=np_bf16)
    # tri[p, j] = 1 if j >= p (valid: sq >= sk within diagonal block)
    tri = (np.arange(128)[None, :] >= np.arange(128)[:, None]).astype(np_bf16)
    tri2 = np.concatenate([tri, tri], axis=1)  # [128, 2*128], per-head copy
    return ka, kb, idm, tri2


def kernel(x, rope_cos, rope_sin, Wq, Wk, Wv, Wo):
    if "nc" not in _CACHE:
        _CACHE["nc"] = build_graph()
    nc = _CACHE["nc"]

    x2 = np.asarray(x, np.float32).reshape(S, D)
    xT = np.ascontiguousarray(x2.T).astype(np_bf16)
    cosT = np.asarray(rope_cos, np.float32).T  # [32, S]
    sinT = np.asarray(rope_sin, np.float32).T
    c2 = np.tile(cosT, (4, 1)).astype(np_bf16)  # [128, S]
    s2 = np.tile(sinT, (4, 1)).astype(np_bf16)
    ka, kb, idm, tri2 = _prep_consts()

    Wq = np.asarray(Wq, np.float32)
    Wk = np.asarray(Wk, np.float32)
    Wv = np.asarray(Wv, np.float32)
    Wo = np.asarray(Wo, np.float32)

    def chunked(w):  # [1024, X] -> [128, 8, X] (partition-major d-chunks)
        return np.ascontiguousarray(
            w.reshape(NDC, 128, -1).transpose(1, 0, 2)
        ).astype(np_bf16)

    wo_b = chunked(Wo)
    in_maps = []
    for c in range(NC_CORES):
        kv = c // 2
        wq_c = chunked(Wq[:, HPC * HD * c : HPC * HD * (c + 1)])
        wkv_c = chunked(
            np.concatenate(
                [Wk[:, HD * kv : HD * (kv + 1)], Wv[:, HD * kv : HD * (kv + 1)]],
                axis=1,
            )
        )
        in_maps.append(
            {
                "xT": xT,
                "wq": wq_c,
                "wkv": wkv_c,
                "wo": wo_b,
                "c2": c2,
                "s2": s2,
                "ka": ka,
                "kb": kb,
                "idm": idm,
                "tri2": tri2,
            }
        )

    res = run_bass_kernel_spmd(nc, in_maps, core_ids=list(range(NC_CORES)))
    out = np.zeros((S, D), np.float32)
    for c in range(NC_CORES):
        blk = np.asarray(res.results[c]["out"], np.float32)
        out[128 * c : 128 * (c + 1)] = blk[0:128]
        out[128 * (8 + c) : 128 * (9 + c)] = blk[128:256]
    return out.reshape(B, S, D)



# revision 9
# speedup vs baseline: 1.1489x; 1.1489x over previous
"""Distributed Trainium2 kernel for GQA attention (nn_Attention_76845554860188).

B=1, S=2048, D=1024, NH=16, NKV=4, HD=64, causal, RoPE, 8 NeuronCores.

Sharding: tensor-parallel over heads. Core c owns q-heads {2c, 2c+1} and their
(shared, GQA) kv-head c//2. Each core projects Q/K/V for all 2048 positions and
runs causal attention for its 2 heads, flash-style: per 512-wide q-chunk, the
PV matmul accumulates in PSUM block-by-block as scores come out of exp, so the
probability tiles stay small and the final chunk's tail is one PV matmul.

Scores use a full-array (128-deep) contraction that folds the q-side RoPE in:
  score = krot.q_rot = [krot; M^T krot] . [q*cos; q*sin]
so q is never explicitly rotated (no q-rope matmuls, no per-head copies) and
the PE array runs with all 128 rows active. The K side builds
kst = [krot; M^T krot] with two small 64-contraction matmuls per seq window
using host-prepared [I | M] / [M^T | I] operators.

Output redistribution uses two AllToAlls on a strided q-block assignment: core
c owns q-128-blocks {c, 8+c}. A2A#1 (blocks 0-7) fires after chunk 1 and
overlaps attention of chunks 2-3 together with the first half of the output
projection; only A2A#2 + the second half-projection sit on the tail.

The softmax denominator comes free as a ones column appended to V in the PV
matmul. exp() runs once per k-block over both heads ([128, 2, w]) on ScalarE
with the 1/sqrt(64) scale folded in; no max-subtraction is needed (logits are
O(5) for unit-scale inputs, far from bf16 overflow).
"""

import sys

sys.path.insert(0, "/opt/trn_rl_repo")

import numpy as np
import ml_dtypes

import concourse.bass as bass
import concourse.mybir as mybir
import concourse.tile as tile
from concourse import bacc
from concourse.bass_utils import run_bass_kernel_spmd

BF16 = mybir.dt.bfloat16
F32 = mybir.dt.float32

B, S, D = 1, 2048, 1024
NH, NKV, HD = 16, 4, 64
NC_CORES = 8
HPC = NH // NC_CORES  # q heads per core = 2
NDC = D // 128  # d chunks = 8
NSB = S // 128  # 128-wide seq blocks = 16
NCH = S // 512  # 512-wide seq chunks = 4
HALF = HD // 2  # 32

np_bf16 = ml_dtypes.bfloat16


def build_graph():
    nc = bacc.Bacc(
        "TRN2", target_bir_lowering=False, debug=False, num_devices=NC_CORES
    )

    # ---- DRAM parameters (per-core shards supplied by host) ----
    xT_e = nc.dram_tensor("xT", [D, S], BF16, kind="ExternalInput")
    wq_e = nc.dram_tensor("wq", [128, NDC, HPC * HD], BF16, kind="ExternalInput")
    wkv_e = nc.dram_tensor("wkv", [128, NDC, 2 * HD], BF16, kind="ExternalInput")
    wo_e = nc.dram_tensor("wo", [128, NDC, D], BF16, kind="ExternalInput")
    c2_e = nc.dram_tensor("c2", [128, S], BF16, kind="ExternalInput")
    s2_e = nc.dram_tensor("s2", [128, S], BF16, kind="ExternalInput")
    ka_e = nc.dram_tensor("ka", [64, 128], BF16, kind="ExternalInput")
    kb_e = nc.dram_tensor("kb", [64, 128], BF16, kind="ExternalInput")
    idm_e = nc.dram_tensor("idm", [128, 128], BF16, kind="ExternalInput")
    tri2_e = nc.dram_tensor("tri2", [128, 2 * 128], BF16, kind="ExternalInput")
    # rows [0:128] = q-block c, rows [128:256] = q-block 8+c
    out_e = nc.dram_tensor("out", [2 * 128, D], BF16, kind="ExternalOutput")

    # A2A bounce buffers: slot j = both 128-wide q-blocks destined for core j
    # ([:, :, 0:128] = q-block j, [:, :, 128:256] = q-block 8+j)
    send_d = nc.dram_tensor("a2a_send", [NC_CORES, 128, 256], BF16)
    recv_d = nc.dram_tensor("a2a_recv", [NC_CORES, 128, 256], BF16)
    # tiny warmup collective: absorbs the entry barrier + collective-stream
    # setup during the preamble so the real A2As run at steady-state cost
    wup_s = nc.dram_tensor("wup_s", [1, 64], BF16)
    wup_r = nc.dram_tensor("wup_r", [NC_CORES, 1, 64], BF16, addr_space="Shared")

    with tile.TileContext(nc) as tc:
        _body(nc, tc, xT_e, wq_e, wkv_e, wo_e, c2_e, s2_e, ka_e, kb_e, idm_e,
              tri2_e, out_e, send_d, recv_d, wup_s, wup_r)

    nc.compile()
    return nc


def _body(nc, tc, xT_e, wq_e, wkv_e, wo_e, c2_e, s2_e, ka_e, kb_e, idm_e,
          tri2_e, out_e, send_d, recv_d, wup_s, wup_r):
    from contextlib import ExitStack

    ctx = ExitStack()
    with ctx:
        consts = ctx.enter_context(tc.tile_pool(name="consts", bufs=1))
        work = ctx.enter_context(tc.tile_pool(name="work", bufs=1))
        rope_cm = tc.tile_pool(name="rope", bufs=1)
        rope = rope_cm.__enter__()
        psum_cm = tc.tile_pool(name="psum", bufs=2, space="PSUM")
        psum = psum_cm.__enter__()

        # warmup collective, first in program order
        wup_sb = consts.tile([1, 64], BF16, tag="wup")
        nc.vector.memset(wup_sb[:], 0.0)
        nc.sync.dma_start(out=wup_s.ap(), in_=wup_sb[:])
        nc.gpsimd.collective_compute(
            "AllGather",
            mybir.AluOpType.bypass,
            replica_groups=[list(range(NC_CORES))],
            ins=[wup_s.ap().opt()],
            outs=[wup_r.ap().opt()],
        )

        # ---- load inputs needed by the preamble ----
        wq_sb = consts.tile([128, NDC, HPC * HD], BF16, tag="wq")
        nc.scalar.dma_start(out=wq_sb[:], in_=wq_e.ap())
        wkv_sb = consts.tile([128, NDC, 2 * HD], BF16, tag="wkv")
        nc.scalar.dma_start(out=wkv_sb[:], in_=wkv_e.ap())
        xT_sb = consts.tile([128, NDC, S], BF16, tag="xT")
        qeng = [nc.sync, nc.scalar, nc.gpsimd]
        for i in range(NDC):
            qeng[i % 3].dma_start(
                out=xT_sb[:, i, :], in_=xT_e[128 * i : 128 * (i + 1), :]
            )
        c2_sb = rope.tile([128, S], BF16, tag="c2")
        nc.sync.dma_start(out=c2_sb[:], in_=c2_e[:, :])
        s2_sb = rope.tile([128, S], BF16, tag="s2")
        nc.scalar.dma_start(out=s2_sb[:], in_=s2_e[:, :])
        ka_sb = rope.tile([64, 128], BF16, tag="ka")
        nc.sync.dma_start(out=ka_sb[:], in_=ka_e[:, :])
        kb_sb = rope.tile([64, 128], BF16, tag="kb")
        nc.sync.dma_start(out=kb_sb[:], in_=kb_e[:, :])
        idm_sb = consts.tile([128, 128], BF16, tag="idm")
        nc.sync.dma_start(out=idm_sb[:], in_=idm_e[:, :])
        tri2_sb = consts.tile([128, 2, 128], BF16, tag="tri2")
        nc.sync.dma_start(
            out=tri2_sb[:], in_=tri2_e.ap().rearrange("p (h n) -> p h n", h=2)
        )

        # ---- Q/KV projections -> PSUM f32 [128, 2048] ----
        # interleaved per d-chunk so both finish right after the last xT DMA
        q_ps = psum.tile([128, S], F32, tag="big")
        kv_ps = psum.tile([128, S], F32, tag="big")
        for i in range(NDC):
            for n in range(NCH):
                nc.tensor.matmul(
                    q_ps[:, 512 * n : 512 * (n + 1)],
                    lhsT=wq_sb[:, i, :],
                    rhs=xT_sb[:, i, 512 * n : 512 * (n + 1)],
                    start=(i == 0),
                    stop=(i == NDC - 1),
                )
            for n in range(NCH):
                nc.tensor.matmul(
                    kv_ps[:, 512 * n : 512 * (n + 1)],
                    lhsT=wkv_sb[:, i, :],
                    rhs=xT_sb[:, i, 512 * n : 512 * (n + 1)],
                    start=(i == 0),
                    stop=(i == NDC - 1),
                )

        # Wo prefetch on the gpsimd DMA queue (idle once xT is in): keeps the
        # late kernel phase DMA-quiet so the A2A rings are free
        wo_sb = consts.tile([128, NDC, D], BF16, tag="wo")
        for i in range(NDC):
            nc.gpsimd.dma_start(out=wo_sb[:, i, :], in_=wo_e[:, i, :])

        # copy projections to SBUF bf16: frees the proj PSUM for kst and the
        # attention pools, and lets the q-side multiplies run in 16-bit DVE
        # mode. qsb (ScalarE) goes first: it releases q_ps, whose PSUM slot
        # the kst accumulator reuses.
        qsb = rope.tile([128, S], BF16, tag="qsb")
        nc.scalar.copy(out=qsb[:], in_=q_ps[:])
        kvsb = rope.tile([128, S], BF16, tag="kvsb")
        nc.scalar.copy(out=kvsb[:], in_=kv_ps[:])

        # ---- K: kst = [krot; M^T krot] via kA = [I | M], kB = [M^T | I] ----
        # kc/ks read kv_ps straight from PSUM so they don't wait on kvsb
        kc_sb = rope.tile([64, S], BF16, tag="kc")
        nc.vector.tensor_tensor(
            out=kc_sb[:], in0=kv_ps[0:64, :], in1=c2_sb[0:64, :],
            op=mybir.AluOpType.mult,
        )
        ks_sb = rope.tile([64, S], BF16, tag="ks")
        nc.vector.tensor_tensor(
            out=ks_sb[:], in0=kv_ps[0:64, :], in1=s2_sb[0:64, :],
            op=mybir.AluOpType.mult,
        )
        kst_ps = psum.tile([128, S], F32, tag="big")
        for n in range(NCH):
            sl = slice(512 * n, 512 * (n + 1))
            nc.tensor.matmul(
                kst_ps[:, sl], lhsT=ka_sb[:], rhs=kc_sb[:, sl],
                start=True, stop=False,
            )
            nc.tensor.matmul(
                kst_ps[:, sl], lhsT=kb_sb[:], rhs=ks_sb[:, sl],
                start=False, stop=True,
            )
        kst_sb = work.tile([128, S], BF16, tag="kst")
        for n in range(NCH):
            sl = slice(512 * n, 512 * (n + 1))
            nc.scalar.copy(out=kst_sb[:, sl], in_=kst_ps[:, sl])

        # ---- q-side RoPE halves: qcs[h] = [q_h * cos; q_h * sin] ----
        # column-halved so chunk-0 scores only wait for the first half
        qcs = work.tile([128, HPC, S], BF16, tag="qcs")

        def qcs_half(cw):
            sl = slice(1024 * cw, 1024 * (cw + 1))
            for h in range(HPC):
                nc.vector.tensor_tensor(
                    out=qcs[0:64, h, sl], in0=qsb[64 * h : 64 * (h + 1), sl],
                    in1=c2_sb[64 * h : 64 * (h + 1), sl],
                    op=mybir.AluOpType.mult,
                )
                nc.vector.tensor_tensor(
                    out=qcs[64:128, h, sl], in0=qsb[64 * h : 64 * (h + 1), sl],
                    in1=s2_sb[64 * h : 64 * (h + 1), sl],
                    op=mybir.AluOpType.mult,
                )

        qcs_half(0)

        # ---- V transpose: kvsb rows 64:128 -> V blocks [128, 64] + ones ----
        vext_sb = work.tile([128, NSB, HD + 1], BF16, tag="vext")
        nc.vector.memset(vext_sb[:, :, HD : HD + 1], 1.0)
        vt_ps = psum.tile([128, NSB, HD], BF16, tag="big")
        for b in range(NSB):
            nc.tensor.transpose(
                vt_ps[:, b, :], kvsb[64:128, 128 * b : 128 * (b + 1)],
                idm_sb[64:128, 64:128],
            )
        nc.vector.tensor_copy(out=vext_sb[:, :, 0:HD], in_=vt_ps[:])
        qcs_half(1)

        # release RoPE temporaries and the projection-phase PSUM pool; the
        # attention phase needs st(4) + ot(2) + op(2) = 8 PSUM banks
        rope_cm.__exit__(None, None, None)
        psum_cm.__exit__(None, None, None)
        ptp = ctx.enter_context(tc.tile_pool(name="pt", bufs=2, space="SBUF"))
        psa_cm = tc.tile_pool(name="psa", bufs=2, space="PSUM")
        psa = psa_cm.__enter__()

        scale = 1.0 / np.sqrt(HD)

        # single receive of the whole A2A result: [128, slot j, 256]
        at_sb = work.tile([128, NC_CORES, 256], BF16, tag="at")

        def oproj(half):
            """Output projection for this core's q-block {c + 8*half}."""
            ou_sb = work.tile([128, D], BF16, tag="ou", bufs=2, name=f"ou{half}")
            for dn in range(2):
                # reuse the ot PSUM ring (free after the kc=3 normalize)
                op_ps = psa.tile([128, 512], F32, tag="ot", bufs=3,
                                 name=f"op{half}_{dn}")
                for j in range(NC_CORES):
                    nc.tensor.matmul(
                        op_ps[:, :],
                        lhsT=at_sb[:, j, 128 * half : 128 * (half + 1)],
                        rhs=wo_sb[:, j, 512 * dn : 512 * (dn + 1)],
                        start=(j == 0),
                        stop=(j == NC_CORES - 1),
                    )
                nc.vector.tensor_copy(
                    out=ou_sb[:, 512 * dn : 512 * (dn + 1)], in_=op_ps[:]
                )
            nc.scalar.dma_start(
                out=out_e.ap()[128 * half : 128 * (half + 1), :], in_=ou_sb[:]
            )

        # ---- attention: flash-style per 512-wide q-chunk. PV trails the
        # scores by one k-block so the Tensor stream never head-of-line
        # blocks on the previous chunk's ot release or the current exp.
        def pv(kc, b, nb, pt):
            cbase = 512 * kc
            q0 = max(cbase, 128 * b)
            w = cbase + 512 - q0
            for h in range(HPC):
                nc.tensor.matmul(
                    ot_h[h][:, q0 - cbase : 512],
                    lhsT=vext_sb[:, b, :],
                    rhs=pt[:, h, 0:w],
                    start=(b == 0),
                    stop=(b == nb - 1),
                )

        for kc in range(NCH):
            cbase = 512 * kc
            nb = 4 * kc + 4  # k-blocks participating in this chunk
            ot_h = [
                psa.tile([HD + 1, 512], F32, tag="ot", bufs=3, name=f"ot{kc}_{h}")
                for h in range(HPC)
            ]
            prev_pt = None
            for b in range(nb):
                q0 = max(cbase, 128 * b)
                w = cbase + 512 - q0
                kb = kst_sb[:, 128 * b : 128 * (b + 1)]
                st_ps = psa.tile([128, 2, 512], F32, tag="st", bufs=2)
                for h in range(HPC):
                    nc.tensor.matmul(
                        st_ps[:, h, 0:w],
                        lhsT=kb,
                        rhs=qcs[:, h, q0 : q0 + w],
                        start=True,
                        stop=True,
                    )
                pt = ptp.tile([128, 2, 512], BF16, tag="pt", bufs=3)
                nc.scalar.activation(
                    out=pt[:, :, 0:w],
                    in_=st_ps[:, :, 0:w],
                    func=mybir.ActivationFunctionType.Exp,
                    scale=scale,
                )
                if 128 * b >= cbase:
                    # diagonal block: mask the leading 128x128 (sq < sk -> 0)
                    nc.vector.tensor_tensor(
                        out=pt[:, :, 0:128],
                        in0=pt[:, :, 0:128],
                        in1=tri2_sb[:],
                        op=mybir.AluOpType.mult,
                    )
                if prev_pt is not None:
                    pv(kc, b - 1, nb, prev_pt)
                prev_pt = pt
            pv(kc, nb - 1, nb, prev_pt)

            # normalize: row HD of ot is the softmax denominator. Per-head
            # chains so h0 starts as soon as its PV accumulation stops.
            stg = work.tile([128, 512], BF16, tag="stg", bufs=2, name=f"stg{kc}")
            for h in range(HPC):
                den_sb = work.tile([1, 512], F32, tag="den", bufs=4)
                nc.vector.tensor_copy(out=den_sb[:], in_=ot_h[h][HD : HD + 1, :])
                rec_sb = work.tile([1, 512], F32, tag="rec", bufs=4)
                nc.vector.reciprocal_approx_fast(out=rec_sb[:], in_=den_sb[:])
                bcr_sb = work.tile([HD, 512], F32, tag="bcr", bufs=4)
                nc.gpsimd.partition_broadcast(bcr_sb[:], rec_sb[:])
                nc.vector.tensor_tensor(
                    out=stg[64 * h : 64 * (h + 1), :],
                    in0=ot_h[h][0:HD, :],
                    in1=bcr_sb[:],
                    op=mybir.AluOpType.mult,
                )
            # stage chunk kc = q-blocks {4kc..4kc+3}: slot qb%8, col-half qb//8
            eng = nc.sync if kc % 2 == 0 else nc.scalar
            eng.dma_start(
                out=send_d.ap()[
                    4 * (kc % 2) : 4 * (kc % 2) + 4,
                    :,
                    128 * (kc // 2) : 128 * (kc // 2) + 128,
                ].rearrange("j p n -> p j n"),
                in_=stg[:],
            )
        # one AllToAll for all four chunks, then receive + both projections
        nc.gpsimd.collective_compute(
            "AllToAll",
            mybir.AluOpType.bypass,
            replica_groups=[list(range(NC_CORES))],
            ins=[send_d.ap().opt()],
            outs=[recv_d.ap().opt()],
        )
        nc.sync.dma_start(
            out=at_sb[:, 0:4, :],
            in_=recv_d.ap()[0:4].rearrange("s p n -> p s n"),
        )
        nc.scalar.dma_start(
            out=at_sb[:, 4:8, :],
            in_=recv_d.ap()[4:8].rearrange("s p n -> p s n"),
        )
        oproj(0)
        oproj(1)

        psa_cm.__exit__(None, None, None)


# ---------------- host side ----------------

_CACHE = {}


def _prep_consts():
    # M: signed half-swap for one 64-wide head (rotate-half convention)
    M = np.zeros((64, 64), np.float32)
    for j in range(HALF):
        M[j, HALF + j] = -1.0
        M[HALF + j, j] = 1.0
    I = np.eye(64, dtype=np.float32)
    # kst = [[I],[M^T]] @ kc + [[M],[I]] @ ks -> lhsT operators [64, 128]
    ka = np.concatenate([I, M], axis=1).astype(np_bf16)
    kb = np.concatenate([M.T, I], axis=1).astype(np_bf16)
    idm = np.eye(128, dtype=np_bf16)
    # tri[p, j] = 1 if j >= p (valid: sq >= sk within diagonal block)
    tri = (np.arange(128)[None, :] >= np.arange(128)[:, None]).astype(np_bf16)
    tri2 = np.concatenate([tri, tri], axis=1)  # [128, 2*128], per-head copy
    return ka, kb, idm, tri2


def kernel(x, rope_cos, rope_sin, Wq, Wk, Wv, Wo):
    if "nc" not in _CACHE:
        _CACHE["nc"] = build_graph()
    nc = _CACHE["nc"]

    x2 = np.asarray(x, np.float32).reshape(S, D)
    xT = np.ascontiguousarray(x2.T).astype(np_bf16)
    cosT = np.asarray(rope_cos, np.float32).T  # [32, S]
    sinT = np.asarray(rope_sin, np.float32).T
    c2 = np.tile(cosT, (4, 1)).astype(np_bf16)  # [128, S]
    s2 = np.tile(sinT, (4, 1)).astype(np_bf16)
    ka, kb, idm, tri2 = _prep_consts()

    Wq = np.asarray(Wq, np.float32)
    Wk = np.asarray(Wk, np.float32)
    Wv = np.asarray(Wv, np.float32)
    Wo = np.asarray(Wo, np.float32)

    def chunked(w):  # [1024, X] -> [128, 8, X] (partition-major d-chunks)
        return np.ascontiguousarray(
            w.reshape(NDC, 128, -1).transpose(1, 0, 2)
        ).astype(np_bf16)

    wo_b = chunked(Wo)
    in_maps = []
    for c in range(NC_CORES):
        kv = c // 2
        wq_c = chunked(Wq[:, HPC * HD * c : HPC * HD * (c + 1)])
        wkv_c = chunked(
            np.concatenate(
                [Wk[:, HD * kv : HD * (kv + 1)], Wv[:, HD * kv : HD * (kv + 1)]],
                axis=1,
            )
        )
        in_maps.append(
            {
                "xT": xT,
                "wq": wq_c,
                "wkv": wkv_c,
                "wo": wo_b,
                "c2": c2,
                "s2": s2,
                "ka": ka,
                "kb": kb,
                "idm": idm,
                "tri2": tri2,
            }
        )

    res = run_bass_kernel_spmd(nc, in_maps, core_ids=list(range(NC_CORES)))
    out = np.zeros((S, D), np.float32)
    for c in range(NC_CORES):
        blk = np.asarray(res.results[c]["out"], np.float32)
        out[128 * c : 128 * (c + 1)] = blk[0:128]
        out[128 * (8 + c) : 128 * (9 + c)] = blk[128:256]
    return out.reshape(B, S, D)

